# revision 1
# baseline (speedup 1.0000x reference)
"""AlgebraicAttention on 8 TRN2 NeuronCores.

Sharding: 8 cores = B(2) x head-groups(4 groups of 4 heads).
Each core: QKV projections for its (b, 4 heads), attention, and a partial
output projection (its 256 Wo rows). Host sums the 4 partials per b and
adds bo. No collectives.

Device-side algebra:
  - K is centered over T before the score matmul, so the score matmul
    directly yields zc = scores - rowmean(scores)  (mean_k q.k_j = q.mean k_j).
  - mad[q] = sum_k |zc[k,q]| via PE ones-matmul reduction (scores are
    computed transposed [k, q] so attn@V and the out-proj need no transposes).
  - s = zc/(|zc| + beta), beta = (mad_mean + 1e-6)/gain  (== y/(|y|+1) with
    y = zc*gain/mad_mean).
  - p = ((s+1)/2)^4 via one fused custom DVE op sq(sq(zb*r*0.5 + 0.5)).
  - Sum_k p comes free from a ones-column appended to V.
  - attn normalization (1/(sum p + 1e-6)) applied to the [65,512] attn@V
    output, not the [T,T] matrix.
  - Biases folded in exactly via an augmented ones-row in x / bias-row in W.
"""

import numpy as np
import ml_dtypes

import concourse.bass as bass
import concourse.tile as tile
from concourse import bacc, mybir
from concourse.bass_utils import run_bass_kernel_spmd

BF16 = mybir.dt.bfloat16
F32 = mybir.dt.float32

T = 2048
C = 1024
NH_TOT = 16
D = 64
NH = 4            # heads per core
CH = NH * D       # 256 channels per core
CIN = 1152        # 1024 + 1 (ones row) padded to 9*128
NKB = T // 128    # 16 k-blocks
NQC = T // 512    # 4 q-chunks
POWER_EPS = 1e-6

_W4 = None


def _get_w4_ops():
    """Register fused custom DVE ops.

    W4:  out = sq(sq(in0*in1*c0 + c1))          (c0=c1=0.5 -> ((s+1)/2)^4)
    W4M: out = sq(sq(in0*in1*c1 + c1)) * (Idx >= c0)   causal-masked variant,
         c0 = per-partition threshold (128*m + r), c1 = 0.5."""
    global _W4
    if _W4 is not None:
        return _W4
    import concourse.dve_ops as dve_ops_mod
    from concourse.dve_spec import Spec, Src0, Src1, C0, C1, Idx, sq, lower
    from concourse.dve_uop import DveOpSpec

    def _ref_w4(in0, in1, s0, s1, imm2):
        a = (in0.astype(np.float32) * in1 * s0 + s1).astype(np.float32)
        a = (a * a).astype(np.float32)
        return (a * a).astype(np.float32)

    def _ref_w4m(in0, in1, s0, s1, imm2):
        a = (in0.astype(np.float32) * in1 * s1 + s1).astype(np.float32)
        a = (a * a).astype(np.float32)
        p = (a * a).astype(np.float32)
        idx = np.arange(in0.shape[-1], dtype=np.float32)
        keep = (idx[None, :] >= np.asarray(s0).reshape(-1, 1)).astype(np.float32)
        return (p * keep.reshape(p.shape[0], *([1] * (p.ndim - 2)), p.shape[-1])).astype(np.float32)

    ops = []
    for name, spec in (
        ("TENSOR_W4_ATTN_ANT",
         Spec(body=sq(sq(Src0 * Src1 * C0 + C1)), reference=_ref_w4)),
        ("TENSOR_W4M_ATTN_ANT",
         Spec(body=sq(sq(Src0 * Src1 * C1 + C1)) * (Idx >= C0),
              reference=_ref_w4m)),
    ):
        if name not in dve_ops_mod._SUB_OPCODE_FOR_NAME:
            row = max(dve_ops_mod._SUB_OPCODE_FOR_NAME.values()) + 1
            assert row < 0x20
            dve_ops_mod._SUB_OPCODE_FOR_NAME[name] = row
        shas = {}
        for ver in ("v3",):
            uops = lower(spec, ver=ver)
            tmp = DveOpSpec(
                name=name,
                opcode=dve_ops_mod.get_dve_sub_opcode(name),
                uops=uops,
                rd1_en=True,
            )
            shas[ver] = tmp.sha(ver)
        op = dve_ops_mod.DveOp(name, spec, subdim=False, uops_sha=shas)
        if all(o.name != name for o in dve_ops_mod.OPS):
            dve_ops_mod.OPS.append(op)
        dve_ops_mod.CUSTOM_DVE_SPECS[name] = spec
        ops.append(op)
    _W4 = tuple(ops)
    return _W4


def _act_raw(nc, out, in_, func, bias=0.0, scale=1.0, accum_out=None):
    """Emit InstActivation directly (also used to bypass the Reciprocal
    ValueError in nc.scalar.activation; LUT accuracy is plenty here)."""
    eng = nc.scalar
    AF = mybir.ActivationFunctionType
    if func not in (AF.Copy, AF.Reciprocal) and not isinstance(bias, bass.AP):
        bias = nc.const_aps.scalar_like(float(bias), in_)
    ins = [eng.lower_ap(in_)]
    for arg in (bias, scale, 0.0):
        if isinstance(arg, bass.AP):
            ins.append(eng.lower_ap(arg))
        else:
            ins.append(mybir.ImmediateValue(dtype=F32, value=float(arg)))
    outs = [eng.lower_ap(out)]
    if accum_out is not None:
        outs.append(eng.lower_ap(accum_out))
    return eng.add_instruction(
        mybir.InstActivation(
            name=nc.get_next_instruction_name(),
            func=func,
            ins=ins,
            outs=outs,
        )
    )


def build_nc(gain: float):
    AF = mybir.ActivationFunctionType
    OP = mybir.AluOpType
    w4op, w4mop = _get_w4_ops()

    nc = bacc.Bacc("TRN2", target_bir_lowering=False, debug=False)

    xt = nc.dram_tensor("xt", [CIN, T], BF16, kind="ExternalInput")
    wq = nc.dram_tensor("wq", [CIN, CH], BF16, kind="ExternalInput")
    wk = nc.dram_tensor("wk", [CIN, CH], BF16, kind="ExternalInput")
    wv = nc.dram_tensor("wv", [CIN, CH], BF16, kind="ExternalInput")
    wo = nc.dram_tensor("wo", [CH, C], BF16, kind="ExternalInput")
    theta = nc.dram_tensor("theta", [128, 4], F32, kind="ExternalInput")
    y = nc.dram_tensor("y", [T, C], F32, kind="ExternalOutput")

    NCB = CIN // 128  # 9 contraction blocks for projections
    inv_mad_scale = 1.0 / (T * gain)
    beta_bias = POWER_EPS / gain

    with tile.TileContext(nc) as tc:
        with tc.tile_pool(name="persist", bufs=1) as persist:
          with tc.tile_pool(name="xw", bufs=1) as xw:
            # ---- load inputs ----
            dmae = [nc.sync, nc.gpsimd, nc.scalar]
            xt_sb = [xw.tile([128, T], BF16, tag=f"xt{i}", name=f"xt{i}") for i in range(NCB)]
            for i in range(NCB):
                dmae[i % 3].dma_start(out=xt_sb[i], in_=xt[i * 128:(i + 1) * 128, :])
            w_sb = {}
            for k, (nm, h) in enumerate((("wq", wq), ("wk", wk), ("wv", wv))):
                w_sb[nm] = [xw.tile([128, CH], BF16, tag=f"{nm}{i}", name=f"{nm}{i}")
                            for i in range(NCB)]
                for i in range(NCB):
                    dmae[(k + i) % 3].dma_start(out=w_sb[nm][i],
                                                in_=h[i * 128:(i + 1) * 128, :])
            wo_sb = [persist.tile([128, C], BF16, tag=f"wo{i}", name=f"wo{i}") for i in range(2)]
            for i in range(2):
                nc.sync.dma_start(out=wo_sb[i], in_=wo[i * 128:(i + 1) * 128, :])
            theta_sb = persist.tile([128, 4], F32, tag="theta", name="theta")
            nc.sync.dma_start(out=theta_sb, in_=theta[:, :])

            ones128 = persist.tile([128, 1], BF16, tag="ones128", name="ones128")
            nc.vector.memset(ones128, 1.0)
            ones1_128 = persist.tile([1, 128], BF16, tag="ones1_128", name="ones1_128")
            nc.vector.memset(ones1_128, 1.0)
            bconst = persist.tile([128, 1], F32, tag="bconst", name="bconst")
            nc.vector.memset(bconst, beta_bias)
            ones1_64 = ones1_128[:, 0:64]

            # persistent activation tensors
            qT = [persist.tile([128, T], BF16, tag=f"qT{i}", name=f"qT{i}") for i in range(2)]
            kcT = [persist.tile([128, T], BF16, tag=f"kcT{i}", name=f"kcT{i}") for i in range(2)]
            v_sb = [persist.tile([128, NKB, 65], BF16, tag=f"v{h}", name=f"v{h}")
                    for h in range(NH)]
            aoT = [persist.tile([128, T], BF16, tag=f"aoT{i}", name=f"aoT{i}") for i in range(2)]

            # ---- projections ----
            with tc.tile_pool(name="ppsum", bufs=6, space="PSUM") as ppsum, \
                 tc.tile_pool(name="pvsum", bufs=2, space="PSUM") as pvsum, \
                 tc.tile_pool(name="ptmp", bufs=4) as ptmp:
                # qT / kT (transposed layout [c, t]), k gets centered
                for nm, dst in (("wk", kcT), ("wq", qT)):
                    ksums = []
                    for co in range(2):
                        acc = ptmp.tile([128, 4], F32, tag="kacc", name="kacc")
                        for tch in range(4):
                            ps = ppsum.tile([128, 512], F32, tag="pj", name="pj")
                            for kb in range(NCB):
                                nc.tensor.matmul(
                                    ps,
                                    lhsT=w_sb[nm][kb][:, co * 128:(co + 1) * 128],
                                    rhs=xt_sb[kb][:, tch * 512:(tch + 1) * 512],
                                    start=(kb == 0), stop=(kb == NCB - 1))
                            if nm == "wk":
                                _act_raw(nc, dst[co][:, tch * 512:(tch + 1) * 512],
                                         ps, AF.Identity,
                                         accum_out=acc[:, tch:tch + 1])
                            else:
                                nc.vector.tensor_copy(
                                    out=dst[co][:, tch * 512:(tch + 1) * 512],
                                    in_=ps)
                        ksums.append(acc)
                    if nm == "wk":
                        for co in range(2):
                            kss = ptmp.tile([128, 1], F32, tag="kss", name="kss")
                            nc.vector.tensor_reduce(
                                out=kss, in_=ksums[co],
                                axis=mybir.AxisListType.X, op=OP.add)
                            nc.scalar.mul(kss, kss, 1.0 / T)
                            nc.vector.tensor_scalar(
                                out=kcT[co], in0=kcT[co],
                                scalar1=kss, scalar2=None, op0=OP.subtract)
                # V in natural layout [t, d], 65th column = 1.0
                for h in range(NH):
                    nc.vector.memset(v_sb[h][:, :, 64:65], 1.0)
                for ti in range(NKB):
                    ps = pvsum.tile([128, 256], F32, tag="pv", name="pv")
                    for kb in range(NCB):
                        nc.tensor.matmul(
                            ps,
                            lhsT=xt_sb[kb][:, ti * 128:(ti + 1) * 128],
                            rhs=w_sb["wv"][kb],
                            start=(kb == 0), stop=(kb == NCB - 1))
                    for h in range(NH):
                        if h % 2 == 0:
                            nc.scalar.copy(v_sb[h][:, ti, 0:64],
                                           ps[:, h * 64:(h + 1) * 64])
                        else:
                            nc.vector.tensor_copy(out=v_sb[h][:, ti, 0:64],
                                                  in_=ps[:, h * 64:(h + 1) * 64])

          # ---- attention (j outer, heads inner; out-proj fused per j) ----
          with tc.tile_pool(name="zb", bufs=2) as zbp, \
               tc.tile_pool(name="tb", bufs=2) as tbp, \
               tc.tile_pool(name="small", bufs=3) as small, \
               tc.tile_pool(name="ysp", bufs=3) as ysp, \
               tc.tile_pool(name="zpsum", bufs=2, space="PSUM") as zpsum, \
               tc.tile_pool(name="madp", bufs=1, space="PSUM") as madp, \
               tc.tile_pool(name="apsum", bufs=2, space="PSUM") as apsum, \
               tc.tile_pool(name="opsum", bufs=1, space="PSUM") as opsum:
              for j in range(NQC):
                  nlow = 4 * j + 4  # blocks at/below diagonal
                  qsl = slice(j * 512, (j + 1) * 512)
                  for hp in range(2):
                    madq = madp.tile([128, 512], F32, tag="madq", name="madq")
                    H = []
                    for hh in range(2):
                      h = hp * 2 + hh
                      co, base = h // 2, (h % 2) * 64
                      H.append(dict(
                          h=h, co=co, base=base,
                          kh=kcT[co][base:base + 64, :],
                          qh=qT[co][base:base + 64, :],
                          zb=zbp.tile([128, NKB, 512], BF16, tag=f"zb{hh}",
                                      name=f"zb{hh}"),
                          tt=tbp.tile([128, NKB, 512], BF16, tag=f"t{hh}",
                                      name=f"t{hh}"),
                          mad=madq[32 * hh:32 * hh + 1, :],
                      ))
                    # stages 1-3 interleaved group-wise: scores -> casts ->
                    # |z| -> mad reduce, per 4-block group, so the mad spine
                    # finishes right after the last score instead of after a
                    # separate full pass. Heads interleaved for HW row/col
                    # tile packing.
                    for g in range(4):
                      for i2 in (2 * g, 2 * g + 1):
                        i0 = 2 * i2
                        for d in H:
                          zps2 = zpsum.tile([128, 2, 512], F32, tag="z", name="z")
                          for di in range(2):
                              i = i0 + di
                              nc.tensor.matmul(
                                  zps2[:, di, :],
                                  lhsT=d["kh"][:, i * 128:(i + 1) * 128],
                                  rhs=d["qh"][:, qsl], start=True, stop=True)
                          if i0 >= nlow:
                              _act_raw(nc, d["tt"][:, i0:i0 + 2, :], zps2,
                                       AF.Abs)
                          elif i2 % 4 == 0:
                              nc.vector.tensor_copy(out=d["zb"][:, i0:i0 + 2, :],
                                                    in_=zps2)
                          else:
                              nc.scalar.copy(d["zb"][:, i0:i0 + 2, :], zps2)
                      if 4 * g < nlow:
                        for d in H:
                          gs = slice(4 * g, 4 * g + 4)
                          nc.vector.tensor_scalar(
                              out=d["tt"][:, gs, :].bitcast(mybir.dt.uint16),
                              in0=d["zb"][:, gs, :].bitcast(mybir.dt.uint16),
                              scalar1=0x7FFF, scalar2=None,
                              op0=OP.bitwise_and)
                      for i in range(4 * g, 4 * g + 4):
                        for hh, d in enumerate(H):
                          nc.tensor.matmul(
                              d["mad"], lhsT=ones128, rhs=d["tt"][:, i, :],
                              start=(i == 0), stop=(i == NKB - 1),
                              tile_position=(0, 32 * hh))
                    # stage 4-7: beta, u, r, p
                    for hh, d in enumerate(H):
                      brow = small.tile([1, 512], BF16, tag=f"brow{hh}",
                                        name=f"brow{hh}")
                      _act_raw(nc, brow, d["mad"], AF.Identity,
                               bias=bconst[0:1, :], scale=inv_mad_scale)
                      bb = small.tile([128, 512], BF16, tag=f"bb{hh}",
                                      name=f"bb{hh}")
                      nc.gpsimd.partition_broadcast(bb, brow, channels=128)
                      d["bb"] = bb
                    for d in H:
                      u = d["tt"][:, 0:nlow, :]
                      bb = d["bb"]
                      bbv = bass.AP(tensor=bb.tensor, offset=bb.offset,
                                    ap=[bb.ap[0], [0, nlow], bb.ap[1]])
                      nc.vector.tensor_tensor(out=u, in0=u, in1=bbv, op=OP.add)
                    for d in H:
                      u = d["tt"][:, 0:nlow, :]
                      _act_raw(nc, u, u, AF.Reciprocal)
                    for d in H:
                      zb, r = d["zb"], d["tt"]
                      if j > 0:
                          nc.vector._custom_dve(
                              w4op, out=zb[:, 0:4 * j, :], in0=zb[:, 0:4 * j, :],
                              in1=r[:, 0:4 * j, :], s0=0.5, s1=0.5)
                      for m in range(4):
                          i = 4 * j + m
                          nc.vector._custom_dve(
                              w4mop, out=zb[:, i, :], in0=zb[:, i, :],
                              in1=r[:, i, :], s0=theta_sb[:, m:m + 1], s1=0.5)
                    # stage 8: attn @ [V | 1] and normalization
                    for d in H:
                      avps = apsum.tile([65, 512], F32, tag="av", name="av")
                      for i in range(nlow):
                          nc.tensor.matmul(
                              avps, lhsT=v_sb[d["h"]][:, i, :],
                              rhs=d["zb"][:, i, :],
                              start=(i == 0), stop=(i == nlow - 1))
                      rrow = small.tile([1, 512], BF16, tag="rrow", name="rrow")
                      _act_raw(nc, rrow, avps[64:65, :], AF.Reciprocal,
                               bias=POWER_EPS)
                      rbb = small.tile([64, 512], BF16, tag="rbbs", name="rbbs")
                      nc.gpsimd.partition_broadcast(rbb, rrow, channels=64)
                      nc.vector.tensor_tensor(
                          out=aoT[d["co"]][d["base"]:d["base"] + 64, qsl],
                          in0=avps[0:64, :], in1=rbb, op=OP.mult)
                  # ---- out-proj for this q-chunk (4 row-blocks) ----
                  for ti in range(4 * j, 4 * j + 4):
                      ys = ysp.tile([128, C], F32, tag="ys", name="ys")
                      for nh2 in range(2):
                          ps = opsum.tile([128, 512], F32, tag="op", name="op")
                          for co2 in range(2):
                              nc.tensor.matmul(
                                  ps, lhsT=aoT[co2][:, ti * 128:(ti + 1) * 128],
                                  rhs=wo_sb[co2][:, nh2 * 512:(nh2 + 1) * 512],
                                  start=(co2 == 0), stop=(co2 == 1))
                          if nh2 == 0:
                              nc.scalar.copy(ys[:, 0:512], ps)
                          else:
                              nc.vector.tensor_copy(out=ys[:, 512:1024], in_=ps)
                      nc.sync.dma_start(out=y[ti * 128:(ti + 1) * 128, :], in_=ys)

    nc.compile()
    return nc


_CACHE = {}


def _bf16(a):
    return np.asarray(a, dtype=ml_dtypes.bfloat16)


def make_in_maps(x, Wq, bq, Wk, bk, Wv, bv, Wo, bo, score_gain,
                 causal_mask):
    x = np.asarray(x, np.float32)

    def aug_w(W, b):
        Wa = np.zeros((CIN, C), np.float32)
        Wa[:C] = np.asarray(W, np.float32)
        Wa[C] = np.asarray(b, np.float32)
        return Wa

    Wqa, Wka, Wva = aug_w(Wq, bq), aug_w(Wk, bk), aug_w(Wv, bv)
    Wof = np.asarray(Wo, np.float32)
    th = (128 * np.arange(4)[None, :] + np.arange(128)[:, None]).astype(np.float32)

    in_maps = []
    for core in range(8):
        b, hg = core // 4, core % 4
        sl = slice(hg * CH, (hg + 1) * CH)
        xa = np.zeros((CIN, T), np.float32)
        xa[:C] = x[b].T
        xa[C] = 1.0
        in_maps.append({
            "xt": _bf16(xa),
            "wq": _bf16(Wqa[:, sl]),
            "wk": _bf16(Wka[:, sl]),
            "wv": _bf16(Wva[:, sl]),
            "wo": _bf16(Wof[sl, :]),
            "theta": th,
        })
    return in_maps


def kernel(x, Wq, bq, Wk, bk, Wv, bv, Wo, bo, score_gain, causal_mask,
           _want_trace=False):
    x = np.asarray(x, np.float32)
    gain = float(np.asarray(score_gain))
    B = x.shape[0]

    key = round(gain, 9)
    if key not in _CACHE:
        _CACHE[key] = build_nc(gain)
    nc = _CACHE[key]

    in_maps = make_in_maps(x=x, Wq=Wq, bq=bq, Wk=Wk, bk=bk, Wv=Wv, bv=bv,
                           Wo=Wo, bo=bo, score_gain=score_gain,
                           causal_mask=causal_mask)

    res = run_bass_kernel_spmd(nc, in_maps, core_ids=list(range(8)),
                               trace=_want_trace)
    out = np.zeros((B, T, C), np.float32)
    for core in range(8):
        out[core // 4] += res.results[core]["y"]
    out += np.asarray(bo, np.float32)
    if _want_trace:
        kernel._last_results = res
    return out

